# revision 11
# baseline (speedup 1.0000x reference)
"""Conv2d 3x3 (stride 1, pad 1, cross-correlation) + scalar bias on 8 TRN2 cores.

Full inputs:  x (32, 128, 56, 56) f32, K (256, 128, 3, 3) f32, bias (1,) f32
Full output:  (32, 256, 56, 56) f32

Sharding: data-parallel over the batch dim — each of the 8 NeuronCores gets 4
images; K and bias are replicated. No collectives needed.

Per-core algorithm (implicit GEMM via shifted matmuls):
  - Host zero-pads each image to 58x58 and lays it out as [Cin=128, 58*58]
    (Cin on SBUF partitions = the matmul contraction dim).
  - For each output row-tile of 8 padded rows (8*58 = 464 moving elements) and
    each Cout chunk of 128, accumulate 9 matmuls in one PSUM bank:
        out[co, p] += K[co, ci, dy, dx] * xpad[ci, p + (dy-1)*58 + (dx-1)]
    lhsT = K slice [ci=128, co=128] (stationary), rhs = shifted xpad slice.
  - Operands are float32r: fp32 bits in memory, PE runs them at full (bf16)
    rate for moving dims >= 256 (walrus requires lhsT/rhs dtypes to match).
  - Input images are loaded in overlapping 10-row halo chunks so the PE can
    start after ~2 chunks instead of after the whole 7 MB input load.
  - PSUM is evacuated through ScalarE activation(Identity, bias=...) which
    folds in the scalar bias, then DMA'd to HBM in a padded 58-wide layout;
    the host strips the 2 padding columns.
"""

import numpy as np

import concourse.tile as tile
import concourse.mybir as mybir
from concourse import bacc
from concourse import bass_utils

N, CIN, H, W = 32, 128, 56, 56
COUT, KH, KW = 256, 3, 3
NCORES = 8
B = N // NCORES            # images per core
HP, WP = H + 2, W + 2      # zero-padded image dims (58x58)
IMG = HP * WP              # 3364
XLEN = IMG + 2             # +1 lead/tail margin so shifted reads stay in-bounds
ROWS = 8                   # output rows per PSUM tile
NT = ROWS * WP             # 464 moving elements per matmul (<= 512 fp32/bank)
NRT = H // ROWS            # 7 row-tiles per image
CHLEN = (ROWS + 2) * WP + 2  # halo chunk: 10 padded rows + 1-elem margins = 582
OWPAD = H * WP             # padded output block per (n, co): 56 rows * 58 cols
GROUP = 4                  # row-tiles evacuated as a batch

F32 = mybir.dt.float32
F32R = mybir.dt.float32r
BF16 = mybir.dt.bfloat16

MM_DTYPE = F32R            # matmul operand dtype (walrus requires lhsT == rhs)

_CACHE = {}


def _build(nreps=1, mm_dtype=None):
    mm_dtype = MM_DTYPE if mm_dtype is None else mm_dtype
    nc = bacc.Bacc("TRN2", target_bir_lowering=False, debug=False)

    x_d = [
        nc.dram_tensor(f"x{n}", [CIN, XLEN], mm_dtype, kind="ExternalInput")
        for n in range(B)
    ]
    kw_d = nc.dram_tensor("kw", [CIN, KH * KW * COUT], mm_dtype, kind="ExternalInput")
    b_d = nc.dram_tensor("biasv", [CIN, 1], F32, kind="ExternalInput")
    y_d = nc.dram_tensor("y", [B, COUT, OWPAD], F32, kind="ExternalOutput")

    with tile.TileContext(nc) as tc:
        rep_ctx = tc.For_i(0, nreps, 1) if nreps > 1 else None
        if rep_ctx is not None:
            rep_ctx.__enter__()
        with (
            tc.tile_pool(name="const", bufs=1) as const,
            tc.tile_pool(name="psum", bufs=8, space="PSUM") as psum,
            tc.tile_pool(name="outs", bufs=6) as outs,
        ):
            # Weights: 18 column-chunk DMAs on the gpsimd queue so the first
            # matmul's weights land fast, in parallel with the x loads.
            kw = const.tile([CIN, KH * KW * COUT], mm_dtype, tag="kw")
            for j in range(2 * KH * KW):
                nc.gpsimd.dma_start(
                    kw[:, j * 128 : (j + 1) * 128], kw_d[:, j * 128 : (j + 1) * 128]
                )
            bias = const.tile([CIN, 1], F32, tag="bias")
            nc.gpsimd.dma_start(bias[:], b_d[:])

            # Input images as overlapping 10-row halo chunks (one tile each) so
            # compute on row-tile i only waits for chunk (n, i).
            xch = {}
            for n in range(B):
                for i in range(NRT):
                    ct = const.tile([CIN, CHLEN], mm_dtype, name="xc", tag=f"x{n}c{i}")
                    nc.sync.dma_start(
                        ct[:], x_d[n][:, i * ROWS * WP : i * ROWS * WP + CHLEN]
                    )
                    xch[(n, i)] = ct

            tiles = [(n, i) for n in range(B) for i in range(NRT)]  # 28 row-tiles
            for g in range(0, len(tiles), GROUP):
                grp = tiles[g : g + GROUP]
                for chunk in range(2):
                    pts = [
                        psum.tile([128, NT], F32, name="pt", tag="pt") for _ in grp
                    ]
                    ki = 0
                    for dy in range(3):
                        for dx in range(3):
                            w = kw[
                                :,
                                (dy * 3 + dx) * COUT + chunk * 128 :
                                (dy * 3 + dx) * COUT + chunk * 128 + 128,
                            ]
                            shift = (dy - 1) * WP + (dx - 1)
                            for t, (n, i) in enumerate(grp):
                                # local offset of output row r=8i+1, col 0 within
                                # the halo chunk: one full row + margin = 59
                                rhs = xch[(n, i)][:, 59 + shift : 59 + shift + NT]
                                nc.tensor.matmul(
                                    pts[t][:], w, rhs, start=(ki == 0), stop=(ki == 8)
                                )
                            ki += 1
                    for t, (n, i) in enumerate(grp):
                        ot = outs.tile([128, NT], F32, name="ot", tag="ot")
                        nc.scalar.activation(
                            ot[:],
                            pts[t][:],
                            mybir.ActivationFunctionType.Identity,
                            bias=bias[:],
                        )
                        nc.sync.dma_start(
                            y_d[
                                n,
                                chunk * 128 : (chunk + 1) * 128,
                                i * ROWS * WP : i * ROWS * WP + NT,
                            ],
                            ot[:],
                        )
        if rep_ctx is not None:
            rep_ctx.__exit__(None, None, None)

    nc.compile()
    return nc


def _get_nc():
    if "nc" not in _CACHE:
        _CACHE["nc"] = _build()
    return _CACHE["nc"]


def _prep_in_maps(x, K, bias, mm_dtype=None):
    mm_dtype = MM_DTYPE if mm_dtype is None else mm_dtype
    np_dt = mybir.dt.np(mm_dtype)
    x = np.ascontiguousarray(x, dtype=np.float32)
    K = np.ascontiguousarray(K, dtype=np.float32)
    bias = np.asarray(bias, dtype=np.float32)

    # kw[ci, (dy*3+dx)*COUT + co] = K[co, ci, dy, dx]
    kw = np.ascontiguousarray(
        K.transpose(1, 2, 3, 0).reshape(CIN, KH * KW * COUT).astype(np_dt)
    )
    biasv = np.full((CIN, 1), bias.reshape(-1)[0], dtype=np.float32)

    # Per-core padded inputs: [CIN, 1 + 58*58 + 1] with zero borders/margins.
    xbuf = np.zeros((NCORES, B, CIN, XLEN), dtype=np_dt)
    view = xbuf[:, :, :, 1 : 1 + IMG].reshape(NCORES, B, CIN, HP, WP)
    view[:, :, :, 1 : 1 + H, 1 : 1 + W] = x.reshape(NCORES, B, CIN, H, W).astype(np_dt)

    in_maps = []
    for c in range(NCORES):
        m = {"kw": kw, "biasv": biasv}
        for n in range(B):
            m[f"x{n}"] = np.ascontiguousarray(xbuf[c, n])
        in_maps.append(m)
    return in_maps


def run_on_cores(x, K, bias, trace=False):
    """Run the SPMD kernel; returns (full_output, BassKernelResults)."""
    nc = _get_nc()
    in_maps = _prep_in_maps(x, K, bias)
    res = bass_utils.run_bass_kernel_spmd(
        nc, in_maps, core_ids=list(range(NCORES)), trace=trace
    )
    out = np.empty((N, COUT, H, W), dtype=np.float32)
    for c in range(NCORES):
        ypad = res.results[c]["y"].reshape(B, COUT, H, WP)
        out[c * B : (c + 1) * B] = ypad[:, :, :, 1 : 1 + W]
    return out, res


def kernel(x, K, bias):
    out, _ = run_on_cores(x, K, bias, trace=False)
    return out


# revision 19
# speedup vs baseline: 1.4494x; 1.4494x over previous
"""Conv2d 3x3 (stride 1, pad 1, cross-correlation) + scalar bias on 8 TRN2 cores.

Full inputs:  x (32, 128, 56, 56) f32, K (256, 128, 3, 3) f32, bias (1,) f32
Full output:  (32, 256, 56, 56) f32

Sharding: data-parallel over the batch dim — each of the 8 NeuronCores gets 4
images; K and bias are replicated. No collectives needed.

Per-core algorithm (implicit GEMM via shifted matmuls):
  - Host zero-pads each image to 58x58 and lays it out as [Cin=128, 58*58]
    (Cin on SBUF partitions = the matmul contraction dim).
  - For each output row-tile of 8 padded rows (8*58 = 464 moving elements) and
    each Cout chunk of 128, accumulate 9 matmuls in one PSUM bank:
        out[co, p] += K[co, ci, dy, dx] * xpad[ci, p + (dy-1)*58 + (dx-1)]
    lhsT = K slice [ci=128, co=128] (stationary), rhs = shifted xpad slice.
  - Operands are float32r: fp32 bits in memory, PE runs them at full (bf16)
    rate for moving dims >= 256 (walrus requires lhsT/rhs dtypes to match).
  - Input images are loaded in overlapping 10-row halo chunks so the PE can
    start after ~2 chunks instead of after the whole 7 MB input load.
  - PSUM is evacuated through ScalarE activation(Identity, bias=...) which
    folds in the scalar bias, then DMA'd to HBM in a padded 58-wide layout;
    the host strips the 2 padding columns.
"""

import numpy as np

import concourse.tile as tile
import concourse.mybir as mybir
from concourse import bacc
from concourse import bass_utils

N, CIN, H, W = 32, 128, 56, 56
COUT, KH, KW = 256, 3, 3
NCORES = 8
B = N // NCORES            # images per core
HP, WP = H + 2, W + 2      # zero-padded image dims (58x58)
IMG = HP * WP              # 3364
XLEN = IMG + 2             # +1 lead/tail margin so shifted reads stay in-bounds
ROWS = 8                   # output rows per PSUM tile
NT = ROWS * WP             # 464 moving elements per matmul (<= 512 fp32/bank)
NRT = H // ROWS            # 7 row-tiles per image
CHLEN = (ROWS + 2) * WP + 2  # halo chunk: 10 padded rows + 1-elem margins = 582
OWPAD = H * WP             # padded output block per (n, co): 56 rows * 58 cols
GROUP = 4                  # row-tiles evacuated as a batch

F32 = mybir.dt.float32
F32R = mybir.dt.float32r
BF16 = mybir.dt.bfloat16

MM_DTYPE = F32R            # matmul operand dtype (walrus requires lhsT == rhs)

_CACHE = {}


def _build(nreps=1, mm_dtype=None):
    mm_dtype = MM_DTYPE if mm_dtype is None else mm_dtype
    nc = bacc.Bacc("TRN2", target_bir_lowering=False, debug=False)

    x_d = [
        nc.dram_tensor(f"x{n}", [CIN, XLEN], mm_dtype, kind="ExternalInput")
        for n in range(B)
    ]
    kw_d = nc.dram_tensor("kw", [CIN, KH * KW * COUT], mm_dtype, kind="ExternalInput")
    b_d = nc.dram_tensor("biasv", [CIN, 1], F32, kind="ExternalInput")
    y_d = nc.dram_tensor("y", [B, COUT, OWPAD], F32, kind="ExternalOutput")

    with tile.TileContext(nc) as tc:
        rep_ctx = tc.For_i(0, nreps, 1) if nreps > 1 else None
        if rep_ctx is not None:
            rep_ctx.__enter__()
        with (
            tc.tile_pool(name="const", bufs=1) as const,
            tc.tile_pool(name="psum", bufs=8, space="PSUM") as psum,
            tc.tile_pool(name="outs", bufs=6) as outs,
        ):
            # PE pre-warm: dummy matmuls on scratch (uninitialized) SBUF while
            # the first input DMAs are in flight, so HAM reaches full clock
            # before the first real matmul.
            wsrc = const.tile([CIN, 640], mm_dtype, tag="warm_src")
            nc.vector.memset(wsrc[:].bitcast(F32), 0.0)
            warm = psum.tile([128, 512], F32, name="warm", tag="pt")
            for _ in range(8):
                nc.tensor.matmul(
                    warm[:], wsrc[:, :128], wsrc[:, 128:640], start=True, stop=True
                )

            # Weights: 18 column-chunk DMAs on the scalar HWDGE queue, issued
            # first so the early matmuls' weights land fast, in parallel with
            # the x loads on the sync queue.
            kw = const.tile([CIN, KH * KW * COUT], mm_dtype, tag="kw")
            for j in range(2 * KH * KW):
                nc.scalar.dma_start(
                    kw[:, j * 128 : (j + 1) * 128], kw_d[:, j * 128 : (j + 1) * 128]
                )
            bias = const.tile([CIN, 1], F32, tag="bias")
            nc.gpsimd.dma_start(bias[:], b_d[:])

            # One SBUF tile per image, filled by 8 disjoint chunk DMAs in
            # consumption order; Tile's subtile dependency tracking lets
            # row-tile i start once its two covering chunks have landed.
            xin = []
            for n in range(B):
                xt = const.tile([CIN, XLEN], mm_dtype, name="xt", tag=f"x{n}")
                for c in range(8):
                    lo = 1 + c * ROWS * WP if c > 0 else 0
                    hi = 1 + (c + 1) * ROWS * WP if c < 7 else XLEN
                    nc.sync.dma_start(xt[:, lo:hi], x_d[n][:, lo:hi])
                xin.append(xt)

            tiles = [(n, i) for n in range(B) for i in range(NRT)]  # 28 row-tiles
            for g in range(0, len(tiles), GROUP):
                grp = tiles[g : g + GROUP]
                for chunk in range(2):
                    pts = [
                        psum.tile([128, NT], F32, name="pt", tag="pt") for _ in grp
                    ]
                    ki = 0
                    for dy in range(3):
                        for dx in range(3):
                            w = kw[
                                :,
                                (dy * 3 + dx) * COUT + chunk * 128 :
                                (dy * 3 + dx) * COUT + chunk * 128 + 128,
                            ]
                            shift = (dy - 1) * WP + (dx - 1)
                            for t, (n, i) in enumerate(grp):
                                # output row r = 8i+1; image data starts at
                                # element 1 of the per-image tile
                                base = 1 + (8 * i + 1) * WP + shift
                                rhs = xin[n][:, base : base + NT]
                                nc.tensor.matmul(
                                    pts[t][:], w, rhs, start=(ki == 0), stop=(ki == 8)
                                )
                            ki += 1
                    for t, (n, i) in enumerate(grp):
                        ot = outs.tile([128, NT], F32, name="ot", tag="ot")
                        # Split PSUM evacuation across ScalarE and VectorE so
                        # bank release (and the kernel tail) isn't serialized
                        # on one engine. Both fold in the scalar bias.
                        if chunk == 0:
                            nc.scalar.activation(
                                ot[:],
                                pts[t][:],
                                mybir.ActivationFunctionType.Identity,
                                bias=bias[:],
                            )
                        else:
                            nc.vector.tensor_scalar_add(ot[:], pts[t][:], bias[:])
                        nc.sync.dma_start(
                            y_d[
                                n,
                                chunk * 128 : (chunk + 1) * 128,
                                i * ROWS * WP : i * ROWS * WP + NT,
                            ],
                            ot[:],
                        )
        if rep_ctx is not None:
            rep_ctx.__exit__(None, None, None)

    nc.compile()
    return nc


def _get_nc():
    if "nc" not in _CACHE:
        _CACHE["nc"] = _build()
    return _CACHE["nc"]


def _prep_in_maps(x, K, bias, mm_dtype=None):
    mm_dtype = MM_DTYPE if mm_dtype is None else mm_dtype
    np_dt = mybir.dt.np(mm_dtype)
    x = np.ascontiguousarray(x, dtype=np.float32)
    K = np.ascontiguousarray(K, dtype=np.float32)
    bias = np.asarray(bias, dtype=np.float32)

    # kw[ci, (dy*3+dx)*COUT + co] = K[co, ci, dy, dx]
    kw = np.ascontiguousarray(
        K.transpose(1, 2, 3, 0).reshape(CIN, KH * KW * COUT).astype(np_dt)
    )
    biasv = np.full((CIN, 1), bias.reshape(-1)[0], dtype=np.float32)

    # Per-core padded inputs: [CIN, 1 + 58*58 + 1] with zero borders/margins.
    xbuf = np.zeros((NCORES, B, CIN, XLEN), dtype=np_dt)
    view = xbuf[:, :, :, 1 : 1 + IMG].reshape(NCORES, B, CIN, HP, WP)
    view[:, :, :, 1 : 1 + H, 1 : 1 + W] = x.reshape(NCORES, B, CIN, H, W).astype(np_dt)

    in_maps = []
    for c in range(NCORES):
        m = {"kw": kw, "biasv": biasv}
        for n in range(B):
            m[f"x{n}"] = np.ascontiguousarray(xbuf[c, n])
        in_maps.append(m)
    return in_maps


def run_on_cores(x, K, bias, trace=False):
    """Run the SPMD kernel; returns (full_output, BassKernelResults)."""
    nc = _get_nc()
    in_maps = _prep_in_maps(x, K, bias)
    res = bass_utils.run_bass_kernel_spmd(
        nc, in_maps, core_ids=list(range(NCORES)), trace=trace
    )
    out = np.empty((N, COUT, H, W), dtype=np.float32)
    for c in range(NCORES):
        ypad = res.results[c]["y"].reshape(B, COUT, H, WP)
        out[c * B : (c + 1) * B] = ypad[:, :, :, 1 : 1 + W]
    return out, res


def kernel(x, K, bias):
    out, _ = run_on_cores(x, K, bias, trace=False)
    return out
